# revision 20
# baseline (speedup 1.0000x reference)
"""Bass/Trainium2 kernel for nn_BilinearPairedLayer.

Math (per batch b):
  xl = concat([x, shift_down(x,1), shift_up(x,1)], -1)      # [N, 192]
  xr = concat([x, shift_up(x,1), shift_down(x,1)], -1)
  hl = relu(xl @ W_l.T + b_l)                               # [N, 128]
  hr = relu(xr @ W_r.T + b_r)
  out[i,j,k] = sum_g (hl @ W_bil[k])[i,g] * hr[j,g] + b_bil[k]   # [N, N, 2]

Sharding: data-parallel over B — core c computes batch b=c (B=8, 8 cores).
The host-side shard step re-lays-out the inputs: x arrives transposed in
bf16 with zero guard columns ([64, 2+1024+2]) and W_l/W_r arrive packed
as per-chunk lhsT tiles [64, 6, 128] bf16, so the device never transposes.

Per-core dataflow (all matmul operands bf16, PSUM accumulate fp32,
output fp32 — rel err ~4e-3, well under the 2e-2 gate):
  - the kernel is output-DMA-bound: 8 MiB of out per core drains at
    ~420 GB/s (~20 us). Everything is organized to issue the first
    512 KiB output DMA as early as possible and keep the drain
    saturated; PE/DVE/ACT have large slack vs the 1.2 us/half drain.
  - bf16 (like fp32r) does not register as PE-busy for the HAM clock
    gate, so plain-fp32 heartbeat matmuls are woven through the kernel
    to reach/hold 2.4 GHz; a warmup spinner fills the pre-input window
    (the PE queue opens ~2 us before the first input DMA lands).
  - input DMAs are packed (W_l+W_r one transfer, both W_bil one
    transfer, b_l+b_r one transfer) and split over the sync/gpsimd
    queues — each DMA completion costs a ~900 ns semaphore hop, so
    fewer, earlier transfers shorten the prefix.
  - the first row-block's serial chain is narrowed: only cols 0-127 of
    hlT/tT are computed before the first out matmul; the remaining 384
    columns backfill during the drain.
  - the context shifts are free: shifted feature chunks of xl^T are
    column-offset views of xT thanks to the guard columns
  - hlT/hrT [128, N] bf16 = relu(W @ xlT + b) via ACT with bias
  - tTk [128, N] bf16 = W_bil[k] (stationary, as stored) @ hlT
  - out tile [i=128, j=512] = tTk[:, iblk].T @ hrT chunk  (PSUM fp32)
  - b_bil + (j,k)-interleave fused into the PSUM->SBUF copy: DVE
    tensor_scalar_add writes k=0 stride-2 columns, ACT activation
    (Identity, per-partition bias) writes k=1 — then a 512 KiB DMA per
    half row-block, alternating sync/gpsimd queues; the final DMA is
    split in quarters so the tail receipt is short.
"""

import numpy as np

B, N, NIN = 8, 1024, 64
H = 128
NOUT = 2
NCH = 512  # matmul free-dim chunk (one PSUM bank of fp32)
GD = 2     # zero guard columns on each side of xT
NWARM = 6

_cached = {}


def _build():
    import concourse.bacc as bacc
    import concourse.mybir as mybir
    import concourse.tile as tile

    f32 = mybir.dt.float32
    bf16 = mybir.dt.bfloat16
    AF = mybir.ActivationFunctionType

    nc = bacc.Bacc("TRN2", target_bir_lowering=False, debug=False, num_devices=8)

    xt_d = nc.dram_tensor("x_t", [NIN, N + 2 * GD], bf16, kind="ExternalInput").ap()
    wlrt_d = nc.dram_tensor("w_lrt", [NIN, 6, H], bf16, kind="ExternalInput").ap()
    blr_d = nc.dram_tensor("b_lr", [H, 2], f32, kind="ExternalInput").ap()
    wb_d = nc.dram_tensor("w_bil", [H, NOUT * H], bf16, kind="ExternalInput").ap()
    bb_d = nc.dram_tensor("b_bil", [NOUT], f32, kind="ExternalInput").ap()
    out_d = nc.dram_tensor("out", [N, N, NOUT], f32, kind="ExternalOutput").ap()
    # row-block view: [8 blocks, 128 rows, (j,k) interleaved 2048]
    out_v = out_d.rearrange("(t p) n k -> t p (n k)", p=128)

    with tile.TileContext(nc) as tc:
        with (
            tc.tile_pool(name="const", bufs=1) as const,
            tc.tile_pool(name="po", bufs=4, space="PSUM") as po_pool,
            tc.tile_pool(name="ps", bufs=3, space="PSUM") as ps,
            tc.tile_pool(name="wps", bufs=1, space="PSUM") as wps_pool,
            tc.tile_pool(name="ob", bufs=10) as ob,
        ):
            # ---- warm tile memset + ACT table preload first: gpsimd's
            # queue opens earliest, so the PE warmup spinner and the
            # scalar table load start right away.
            warm = const.tile([128, 128], f32)
            nc.gpsimd.memset(warm, 0.0)
            # dummy ACT ops on a scratch tile: pull the lazy ACT table load
            # to the front without adding deps on `warm`
            actscratch = const.tile([1, 4], f32)
            nc.scalar.activation(actscratch[0:1, 0:2], warm[0:1, 0:2], AF.Relu)
            nc.scalar.activation(actscratch[0:1, 2:4], warm[0:1, 0:2], AF.Identity)

            # ---- input DMAs: x halves on sync, packed W_l/W_r first on
            # gpsimd, then W_bil/biases
            XSPLIT = NCH + 2 * GD
            xT = const.tile([NIN, N + 2 * GD], bf16)
            nc.sync.dma_start(out=xT[:, 0:XSPLIT], in_=xt_d[:, 0:XSPLIT])
            wlrT = const.tile([NIN, 6, H], bf16)
            nc.gpsimd.dma_start(out=wlrT, in_=wlrt_d)
            nc.sync.dma_start(out=xT[:, XSPLIT:], in_=xt_d[:, XSPLIT:])
            wb = const.tile([H, NOUT * H], bf16)
            nc.gpsimd.dma_start(out=wb, in_=wb_d)
            blr_s = const.tile([H, 2], f32)
            nc.gpsimd.dma_start(out=blr_s, in_=blr_d)
            bb_s = const.tile([128, NOUT], f32)
            nc.gpsimd.dma_start(
                out=bb_s, in_=bb_d.unsqueeze(0).broadcast_to([128, NOUT])
            )
            wlT = wlrT[:, 0:3, :]
            wrT = wlrT[:, 3:6, :]
            wb0 = wb[:, 0:H]
            wb1 = wb[:, H : 2 * H]

            # ---- PE heartbeats: plain fp32 counts as PE-busy for the
            # HAM clock gate (bf16/f32r do not); ~5 us of accumulated
            # fp32 busy earns the 2.4 GHz boost. They are emitted at the
            # END of the program (lowest priority) so the ready-first
            # scheduler runs them ONLY when the PE is otherwise idle —
            # the pre-input window and chain stalls — never delaying
            # real work.
            wps = wps_pool.tile([128, NCH], f32, tag="warm")

            def heartbeat():
                nc.tensor.matmul(
                    wps[:, 0:128], warm, warm,
                    start=True, stop=True, skip_group_check=True,
                )

            # explicit early warmup: spins the PE from queue-open until
            # the input DMAs land, starting the fp32 boost-credit clock
            for _ in range(NWARM):
                heartbeat()

            hlT = const.tile([H, N], bf16)
            hrT = const.tile([H, N], bf16)

            def h_cols(dst, wt, bias, s1, j0, w, split=False):
                # chunk 1 is shift_down (src col i-1) for xl, shift_up (i+1) for xr
                ph = ps.tile([128, NCH], f32, tag="ps")
                for c, s in ((0, 0), (1, s1), (2, -s1)):
                    nc.tensor.matmul(
                        ph[:, 0:w],
                        wt[:, c, :],
                        xT[:, GD + j0 + s : GD + j0 + s + w],
                        start=(c == 0), stop=(c == 2),
                    )
                halves = ((0, w // 2), (w // 2, w)) if split else ((0, w),)
                for lo, hi in halves:
                    nc.scalar.activation(
                        dst[:, j0 + lo : j0 + hi], ph[:, lo:hi], AF.Relu,
                        bias=bias, scale=1.0,
                    )

            tT0 = const.tile([H, N], bf16)
            tT1 = const.tile([H, N], bf16)

            def t_cols(wbk, tT, j0, w, on_act=False):
                pt = ps.tile([128, NCH], f32, tag="ps")
                nc.tensor.matmul(
                    pt[:, 0:w], wbk, hlT[:, j0 : j0 + w], start=True, stop=True
                )
                if on_act:
                    nc.scalar.copy(tT[:, j0 : j0 + w], pt[:, 0:w])
                else:
                    nc.vector.tensor_copy(tT[:, j0 : j0 + w], pt[:, 0:w])

            _dmaq = [0]

            def out_half(iblk, j0, last=False, nsplit=1, hb=False):
                if hb:
                    heartbeat()
                # nsplit>1: pipeline the half in nsplit j-strips — each
                # strip is mm -> interleave -> DMA, shortening the first
                # byte-to-HBM latency (used for the opening halves).
                ohalf = ob.tile([128, 2 * NCH], f32, tag="ob")
                w = NCH // nsplit
                pos = []
                for k, tT in ((0, tT0), (1, tT1)):
                    po = po_pool.tile([128, NCH], f32, tag="po")
                    pos.append(po)
                    for s in range(nsplit):
                        nc.tensor.matmul(
                            po[:, s * w : (s + 1) * w],
                            tT[:, iblk * 128 : (iblk + 1) * 128],
                            hrT[:, j0 + s * w : j0 + (s + 1) * w],
                            start=True, stop=True,
                        )
                        dst = ohalf[:, 2 * s * w + k : 2 * (s + 1) * w : 2]
                        src = po[:, s * w : (s + 1) * w]
                        if k == 0:
                            nc.vector.tensor_scalar_add(dst, src, bb_s[:, 0:1])
                        else:
                            nc.scalar.activation(
                                dst, src, AF.Identity, bias=bb_s[:, 1:2], scale=1.0
                            )
                dst_v = out_v[iblk][:, 2 * j0 : 2 * j0 + 2 * NCH]
                nd = 4 if last else nsplit
                q = 2 * NCH // nd
                for qi in range(nd):
                    eng = nc.sync if (_dmaq[0] + qi) % 2 == 0 else nc.gpsimd
                    nc_dst = dst_v[:, qi * q : (qi + 1) * q]
                    eng.dma_start(out=nc_dst, in_=ohalf[:, qi * q : (qi + 1) * q])
                _dmaq[0] += 1

            # emission order = scheduler priority: shortest serial chain
            # to the first output DMA (narrow 128-col hl/t prefix for
            # iblk 0), then the backfill in small bands between the early
            # out halves — the ready-first list scheduler slots backfill
            # into engine idle time without delaying the drain.
            bl = blr_s[:, 0:1]
            br = blr_s[:, 1:2]
            h_cols(hlT, wlT, bl, -1, 0, 128)          # hl cols 0-127
            h_cols(hrT, wrT, br, +1, 0, NCH)          # hr cols 0-511
            t_cols(wb0, tT0, 0, 128)
            t_cols(wb1, tT1, 0, 128, on_act=True)
            out_half(0, 0)
            # i-strips 1..7: each 128-col hl/t strip unlocks its own
            # out-half immediately, pipelining across PE/ACT/DVE
            for s in range(1, 8):
                j = s * 128
                h_cols(hlT, wlT, bl, -1, j, 128)
                t_cols(wb0, tT0, j, 128)
                t_cols(wb1, tT1, j, 128, on_act=True)
                out_half(s, 0, hb=(s <= 6))
            h_cols(hrT, wrT, br, +1, NCH, NCH)        # hr cols 512-1023
            for iblk in range(8):
                out_half(iblk, NCH, last=(iblk == 7))

    nc.finalize()
    return nc


def make_in_maps(x_l, W_l, b_l, W_r, b_r, W_bil, b_bil):
    # host-side layout: W_l/W_r packed to lhsT chunks [f=64, 6, h] bf16,
    # x to [64, N] bf16 with zero guard columns, W_bil to [h, k*g] bf16
    import ml_dtypes

    bf = ml_dtypes.bfloat16

    def w_chunks(W):
        return np.asarray(W, np.float32).reshape(H, 3, NIN).transpose(2, 1, 0)

    wlr = np.concatenate([w_chunks(W_l), w_chunks(W_r)], axis=1).astype(bf)

    x_l = np.asarray(x_l, np.float32)
    xt = np.zeros((B, NIN, N + 2 * GD), bf)
    xt[:, :, GD : GD + N] = x_l.transpose(0, 2, 1).astype(bf)

    wb = np.asarray(W_bil, np.float32).transpose(1, 0, 2).reshape(H, NOUT * H)

    com = {
        "w_lrt": np.ascontiguousarray(wlr),
        "b_lr": np.ascontiguousarray(
            np.stack([np.asarray(b_l, np.float32), np.asarray(b_r, np.float32)], 1)
        ),
        "w_bil": np.ascontiguousarray(wb).astype(bf),
        "b_bil": np.ascontiguousarray(b_bil, np.float32),
    }
    return [{"x_t": np.ascontiguousarray(xt[c]), **com} for c in range(B)]


def kernel(x_l, W_l, b_l, W_r, b_r, W_bil, b_bil):
    from concourse import bass_utils

    if "nc" not in _cached:
        _cached["nc"] = _build()
    nc = _cached["nc"]

    in_maps = make_in_maps(x_l, W_l, b_l, W_r, b_r, W_bil, b_bil)
    res = bass_utils.run_bass_kernel_spmd(nc, in_maps, core_ids=list(range(B)))
    return np.stack([res.results[c]["out"] for c in range(B)], axis=0)


# revision 21
# speedup vs baseline: 1.0379x; 1.0379x over previous
"""Bass/Trainium2 kernel for nn_BilinearPairedLayer.

Math (per batch b):
  xl = concat([x, shift_down(x,1), shift_up(x,1)], -1)      # [N, 192]
  xr = concat([x, shift_up(x,1), shift_down(x,1)], -1)
  hl = relu(xl @ W_l.T + b_l)                               # [N, 128]
  hr = relu(xr @ W_r.T + b_r)
  out[i,j,k] = sum_g (hl @ W_bil[k])[i,g] * hr[j,g] + b_bil[k]   # [N, N, 2]

Sharding: data-parallel over B — core c computes batch b=c (B=8, 8 cores).
The host-side shard step re-lays-out the inputs: x arrives transposed in
bf16 with zero guard columns ([64, 2+1024+2]) and W_l/W_r arrive packed
as per-chunk lhsT tiles [64, 6, 128] bf16, so the device never transposes.

Per-core dataflow (all matmul operands bf16, PSUM accumulate fp32,
output fp32 — rel err ~4e-3, well under the 2e-2 gate):
  - the kernel is output-DMA-bound: 8 MiB of out per core drains at
    ~420 GB/s (~20 us). Everything is organized to issue the first
    512 KiB output DMA as early as possible and keep the drain
    saturated; PE/DVE/ACT have large slack vs the 1.2 us/half drain.
  - bf16 (like fp32r) does not register as PE-busy for the HAM clock
    gate, so plain-fp32 heartbeat matmuls are woven through the kernel
    to reach/hold 2.4 GHz; a warmup spinner fills the pre-input window
    (the PE queue opens ~2 us before the first input DMA lands).
  - input DMAs are packed (W_l+W_r one transfer, both W_bil one
    transfer, b_l+b_r one transfer) and split over the sync/gpsimd
    queues — each DMA completion costs a ~900 ns semaphore hop, so
    fewer, earlier transfers shorten the prefix.
  - the first row-block's serial chain is narrowed: only cols 0-127 of
    hlT/tT are computed before the first out matmul; the remaining 384
    columns backfill during the drain.
  - the context shifts are free: shifted feature chunks of xl^T are
    column-offset views of xT thanks to the guard columns
  - hlT/hrT [128, N] bf16 = relu(W @ xlT + b) via ACT with bias
  - tTk [128, N] bf16 = W_bil[k] (stationary, as stored) @ hlT
  - out tile [i=128, j=512] = tTk[:, iblk].T @ hrT chunk  (PSUM fp32)
  - b_bil + (j,k)-interleave fused into the PSUM->SBUF copy: DVE
    tensor_scalar_add writes k=0 stride-2 columns, ACT activation
    (Identity, per-partition bias) writes k=1 — then a 512 KiB DMA per
    half row-block, alternating sync/gpsimd queues; the final DMA is
    split in quarters so the tail receipt is short.
"""

import numpy as np

B, N, NIN = 8, 1024, 64
H = 128
NOUT = 2
NCH = 512  # matmul free-dim chunk (one PSUM bank of fp32)
GD = 2     # zero guard columns on each side of xT
NWARM = 6

_cached = {}


def _build():
    import concourse.bacc as bacc
    import concourse.mybir as mybir
    import concourse.tile as tile

    f32 = mybir.dt.float32
    bf16 = mybir.dt.bfloat16
    AF = mybir.ActivationFunctionType

    nc = bacc.Bacc("TRN2", target_bir_lowering=False, debug=False, num_devices=8)

    xt_d = nc.dram_tensor("x_t", [NIN, N + 2 * GD], bf16, kind="ExternalInput").ap()
    wlrt_d = nc.dram_tensor("w_lrt", [NIN, 6, H], bf16, kind="ExternalInput").ap()
    blr_d = nc.dram_tensor("b_lr", [H, 2], f32, kind="ExternalInput").ap()
    wb_d = nc.dram_tensor("w_bil", [H, NOUT * H], bf16, kind="ExternalInput").ap()
    bb_d = nc.dram_tensor("b_bil", [NOUT], f32, kind="ExternalInput").ap()
    out_d = nc.dram_tensor("out", [N, N, NOUT], f32, kind="ExternalOutput").ap()
    # row-block view: [8 blocks, 128 rows, (j,k) interleaved 2048]
    out_v = out_d.rearrange("(t p) n k -> t p (n k)", p=128)

    with tile.TileContext(nc) as tc:
        with (
            tc.tile_pool(name="const", bufs=1) as const,
            tc.tile_pool(name="po", bufs=4, space="PSUM") as po_pool,
            tc.tile_pool(name="ps", bufs=3, space="PSUM") as ps,
            tc.tile_pool(name="wps", bufs=1, space="PSUM") as wps_pool,
            tc.tile_pool(name="ob", bufs=10) as ob,
        ):
            # ---- warm tile memset + ACT table preload first: gpsimd's
            # queue opens earliest, so the PE warmup spinner and the
            # scalar table load start right away.
            warm = const.tile([128, 128], f32)
            nc.gpsimd.memset(warm, 0.0)
            # dummy ACT ops on a scratch tile: pull the lazy ACT table load
            # to the front without adding deps on `warm`
            actscratch = const.tile([1, 4], f32)
            nc.scalar.activation(actscratch[0:1, 0:2], warm[0:1, 0:2], AF.Relu)
            nc.scalar.activation(actscratch[0:1, 2:4], warm[0:1, 0:2], AF.Identity)

            # ---- input DMAs: x halves on sync, packed W_l/W_r first on
            # gpsimd, then W_bil/biases
            XSPLIT = NCH + 2 * GD
            xT = const.tile([NIN, N + 2 * GD], bf16)
            nc.sync.dma_start(out=xT[:, 0:XSPLIT], in_=xt_d[:, 0:XSPLIT])
            wlrT = const.tile([NIN, 6, H], bf16)
            nc.gpsimd.dma_start(out=wlrT, in_=wlrt_d)
            nc.sync.dma_start(out=xT[:, XSPLIT:], in_=xt_d[:, XSPLIT:])
            wb = const.tile([H, NOUT * H], bf16)
            nc.gpsimd.dma_start(out=wb, in_=wb_d)
            blr_s = const.tile([H, 2], f32)
            nc.gpsimd.dma_start(out=blr_s, in_=blr_d)
            bb_s = const.tile([128, NOUT], f32)
            nc.gpsimd.dma_start(
                out=bb_s, in_=bb_d.unsqueeze(0).broadcast_to([128, NOUT])
            )
            wlT = wlrT[:, 0:3, :]
            wrT = wlrT[:, 3:6, :]
            wb0 = wb[:, 0:H]
            wb1 = wb[:, H : 2 * H]

            # ---- PE heartbeats: plain fp32 counts as PE-busy for the
            # HAM clock gate (bf16/f32r do not); ~5 us of accumulated
            # fp32 busy earns the 2.4 GHz boost. They are emitted at the
            # END of the program (lowest priority) so the ready-first
            # scheduler runs them ONLY when the PE is otherwise idle —
            # the pre-input window and chain stalls — never delaying
            # real work.
            wps = wps_pool.tile([128, NCH], f32, tag="warm")

            def heartbeat():
                nc.tensor.matmul(
                    wps[:, 0:128], warm, warm,
                    start=True, stop=True, skip_group_check=True,
                )

            # explicit early warmup: spins the PE from queue-open until
            # the input DMAs land, starting the fp32 boost-credit clock
            for _ in range(NWARM):
                heartbeat()

            hlT = const.tile([H, N], bf16)
            hrT = const.tile([H, N], bf16)

            def h_cols(dst, wt, bias, s1, j0, w, split=False):
                # chunk 1 is shift_down (src col i-1) for xl, shift_up (i+1) for xr
                ph = ps.tile([128, NCH], f32, tag="ps")
                for c, s in ((0, 0), (1, s1), (2, -s1)):
                    nc.tensor.matmul(
                        ph[:, 0:w],
                        wt[:, c, :],
                        xT[:, GD + j0 + s : GD + j0 + s + w],
                        start=(c == 0), stop=(c == 2),
                    )
                halves = ((0, w // 2), (w // 2, w)) if split else ((0, w),)
                for lo, hi in halves:
                    nc.scalar.activation(
                        dst[:, j0 + lo : j0 + hi], ph[:, lo:hi], AF.Relu,
                        bias=bias, scale=1.0,
                    )

            tT0 = const.tile([H, N], bf16)
            tT1 = const.tile([H, N], bf16)

            def t_cols(wbk, tT, j0, w, on_act=False):
                pt = ps.tile([128, NCH], f32, tag="ps")
                nc.tensor.matmul(
                    pt[:, 0:w], wbk, hlT[:, j0 : j0 + w], start=True, stop=True
                )
                if on_act:
                    nc.scalar.copy(tT[:, j0 : j0 + w], pt[:, 0:w])
                else:
                    nc.vector.tensor_copy(tT[:, j0 : j0 + w], pt[:, 0:w])

            _dmaq = [0]

            def out_half(iblk, j0, last=False, nsplit=1, hb=False):
                if hb:
                    heartbeat()
                # nsplit>1: pipeline the half in nsplit j-strips — each
                # strip is mm -> interleave -> DMA, shortening the first
                # byte-to-HBM latency (used for the opening halves).
                ohalf = ob.tile([128, 2 * NCH], f32, tag="ob")
                w = NCH // nsplit
                pos = []
                for k, tT in ((0, tT0), (1, tT1)):
                    po = po_pool.tile([128, NCH], f32, tag="po")
                    pos.append(po)
                    for s in range(nsplit):
                        nc.tensor.matmul(
                            po[:, s * w : (s + 1) * w],
                            tT[:, iblk * 128 : (iblk + 1) * 128],
                            hrT[:, j0 + s * w : j0 + (s + 1) * w],
                            start=True, stop=True,
                        )
                        dst = ohalf[:, 2 * s * w + k : 2 * (s + 1) * w : 2]
                        src = po[:, s * w : (s + 1) * w]
                        if k == 0:
                            nc.vector.tensor_scalar_add(dst, src, bb_s[:, 0:1])
                        else:
                            nc.scalar.activation(
                                dst, src, AF.Identity, bias=bb_s[:, 1:2], scale=1.0
                            )
                dst_v = out_v[iblk][:, 2 * j0 : 2 * j0 + 2 * NCH]
                nd = 4 if last else nsplit
                q = 2 * NCH // nd
                for qi in range(nd):
                    eng = nc.sync if (_dmaq[0] + qi) % 2 == 0 else nc.gpsimd
                    nc_dst = dst_v[:, qi * q : (qi + 1) * q]
                    eng.dma_start(out=nc_dst, in_=ohalf[:, qi * q : (qi + 1) * q])
                _dmaq[0] += 1

            # emission order = scheduler priority: shortest serial chain
            # to the first output DMA (narrow 128-col hl/t prefix for
            # iblk 0), then the backfill in small bands between the early
            # out halves — the ready-first list scheduler slots backfill
            # into engine idle time without delaying the drain.
            bl = blr_s[:, 0:1]
            br = blr_s[:, 1:2]
            h_cols(hlT, wlT, bl, -1, 0, 128)          # hl cols 0-127
            h_cols(hrT, wrT, br, +1, 0, NCH)          # hr cols 0-511
            t_cols(wb0, tT0, 0, 128)
            t_cols(wb1, tT1, 0, 128, on_act=True)
            out_half(0, 0)
            h_cols(hlT, wlT, bl, -1, 128, 384)        # hl cols 128-511
            t_cols(wb0, tT0, 128, 384)
            t_cols(wb1, tT1, 128, 384, on_act=True)
            out_half(1, 0, hb=True)
            h_cols(hlT, wlT, bl, -1, NCH, NCH)        # hl cols 512-1023
            out_half(2, 0, hb=True)
            t_cols(wb0, tT0, NCH, NCH)
            t_cols(wb1, tT1, NCH, NCH, on_act=True)
            out_half(3, 0, hb=True)
            h_cols(hrT, wrT, br, +1, NCH, NCH)        # hr cols 512-1023
            for iblk in range(4, 8):
                out_half(iblk, 0, hb=(iblk < 7))
            for iblk in range(4):
                out_half(iblk, NCH)
            for iblk in range(4, 8):
                out_half(iblk, NCH, last=(iblk == 7))

    nc.finalize()
    return nc


def make_in_maps(x_l, W_l, b_l, W_r, b_r, W_bil, b_bil):
    # host-side layout: W_l/W_r packed to lhsT chunks [f=64, 6, h] bf16,
    # x to [64, N] bf16 with zero guard columns, W_bil to [h, k*g] bf16
    import ml_dtypes

    bf = ml_dtypes.bfloat16

    def w_chunks(W):
        return np.asarray(W, np.float32).reshape(H, 3, NIN).transpose(2, 1, 0)

    wlr = np.concatenate([w_chunks(W_l), w_chunks(W_r)], axis=1).astype(bf)

    x_l = np.asarray(x_l, np.float32)
    xt = np.zeros((B, NIN, N + 2 * GD), bf)
    xt[:, :, GD : GD + N] = x_l.transpose(0, 2, 1).astype(bf)

    wb = np.asarray(W_bil, np.float32).transpose(1, 0, 2).reshape(H, NOUT * H)

    com = {
        "w_lrt": np.ascontiguousarray(wlr),
        "b_lr": np.ascontiguousarray(
            np.stack([np.asarray(b_l, np.float32), np.asarray(b_r, np.float32)], 1)
        ),
        "w_bil": np.ascontiguousarray(wb).astype(bf),
        "b_bil": np.ascontiguousarray(b_bil, np.float32),
    }
    return [{"x_t": np.ascontiguousarray(xt[c]), **com} for c in range(B)]


def kernel(x_l, W_l, b_l, W_r, b_r, W_bil, b_bil):
    from concourse import bass_utils

    if "nc" not in _cached:
        _cached["nc"] = _build()
    nc = _cached["nc"]

    in_maps = make_in_maps(x_l, W_l, b_l, W_r, b_r, W_bil, b_bil)
    res = bass_utils.run_bass_kernel_spmd(nc, in_maps, core_ids=list(range(B)))
    return np.stack([res.results[c]["out"] for c in range(B)], axis=0)
